# revision 25
# baseline (speedup 1.0000x reference)
"""3-layer GCN (GCNConv x3 + LeakyReLU, PyG semantics) on 8 Trainium2 cores.

Strategy (source-partitioned, ReduceScatter):
  - Core c owns nodes [c*NPC, (c+1)*NPC) and processes the edges whose SOURCE
    it owns.  Self loops are added as explicit edges, so a whole layer is
        OUT[d] = lrelu(dinv[d] * sum_{e: dst=d} G[src_e] + bias),
    with G = dinv * (H @ W) computed locally per core (no feature exchange).
  - Per layer: phase A computes G for own nodes into two local DRAM gather
    tables (low/high half of own rows, int16-indexable, 256B rows), then
    dma_gather over globally dst-sorted edges + one-hot matmul segment-sum
    into PSUM per global dst tile (784 tiles across all cores), converted to
    bf16 and written into a row-major [100352, 64] partial-sum table.
  - One ReduceScatter(add) delivers each core the full sum for its own
    12544 rows -- an output-sized collective (~57us) instead of AllGathering
    the whole feature table (~284us).
  - Epilogue per own tile: scale by dinv (ACT), add bias (DVE), LeakyReLU
    (ACT), PE transpose, and the next layer's phase-A matmul (bf16).
  - One-hot q matrices (is_equal(iota, drel)) are split across DVE and Pool;
    PSUM->bf16 converts across ACT/DVE, keeping every engine below the DMA
    roofline.  Gathers use one large call per (gather-group, table-half)
    span to amortize the ~1us SWDGE fixed cost per call.

The Bass program is SPMD: one program, per-core data.  Section lengths are
shared across cores (max over cores, padded with dummy edges whose one-hot
column is zero: dstrel = -1).
"""
import sys

sys.path.insert(0, "/opt/trn_rl_repo")

import numpy as np

import concourse.bacc as bacc
import concourse.mybir as mybir
import concourse.tile as tile
from concourse import library_config
from concourse.bass_utils import run_bass_kernel_spmd
from concourse.masks import make_identity

_F32 = mybir.dt.float32
_BF16 = mybir.dt.bfloat16
_I16 = mybir.dt.int16
P = 128
D = 64
NEG_SLOPE = 0.01


class Cfg:
    def __init__(self, n_nodes=100000, cores=8):
        self.N = n_nodes
        self.CORES = cores
        self.NPC = self.N // cores            # nodes owned per core
        self.TILES = (self.NPC + P - 1) // P  # own-node tiles per core (98)
        self.RPC = self.TILES * P             # padded rows per core (12544)
        self.GR = cores * self.RPC            # global padded rows (100352)
        self.GTILES = cores * self.TILES      # global dst tiles (784)
        self.KB = 1                           # source buckets (gather tables)
        assert self.RPC % self.KB == 0
        self.HALF = self.RPC // self.KB       # rows per gather table (12544)
        assert self.HALF <= 32767
        self.GGT = 28                         # dst tiles per gather group
        assert self.GTILES % self.GGT == 0
        self.NGG = self.GTILES // self.GGT    # gather groups (28)
        self.GT = 7                           # dst tiles per write group
        assert self.GGT % self.GT == 0
        self.WPG = self.GGT // self.GT        # write groups per gather group
        self.HTILES = self.TILES // self.KB   # own tiles per table (98)
        assert self.TILES % self.KB == 0
        self.LAYERS = 3
        self.MSGBUFS = 3
        self.QBUFS = 12
        self.PSABUFS = 3
        # engine split knobs
        self.Q_SPLIT = 1000000      # every Q_SPLIT-th q op goes to Pool
        self.CONV_MOD = 1000000     # converts: i%MOD==0 -> DVE, else ACT


DEFAULT_CFG = Cfg()


def _preprocess(edge_index, cfg):
    """Sort/pack edges; build per-core device arrays and shared metadata."""
    src0 = np.asarray(edge_index[0], dtype=np.int64)
    dst0 = np.asarray(edge_index[1], dtype=np.int64)
    N, CORES, NPC, TILES = cfg.N, cfg.CORES, cfg.NPC, cfg.TILES
    NGG, KB, GGT, HALF = cfg.NGG, cfg.KB, cfg.GGT, cfg.HALF

    src = src0
    dst = dst0

    deg = np.bincount(dst, minlength=N).astype(np.float32) + 1.0
    dinv = (1.0 / np.sqrt(deg)).astype(np.float32)

    owner = src // NPC                      # processing core (source owner)
    slocal = src % NPC
    bucket = slocal // HALF                 # gather-table half
    lrow = (slocal % HALF).astype(np.int16)

    downer = dst // NPC
    dloc = dst % NPC
    otile = dloc // P                       # owner-local tile 0..97
    HT2 = TILES // 2
    # (half, owner, tile-in-half): ReduceScatter halves then split per owner
    gtile = (otile // HT2) * (CORES * HT2) + downer * HT2 + otile % HT2
    drel_v = (dloc % P).astype(np.float32)
    grp = gtile // GGT
    tl = gtile % GGT

    counts = np.zeros((CORES, NGG, KB, GGT), dtype=np.int64)
    np.add.at(counts, (owner, grp, bucket, tl), 1)
    order = np.lexsort((tl, bucket, grp, owner))
    s_lrow = lrow[order]
    s_drel = drel_v[order]

    sec_len = counts.max(axis=0)            # [NGG, KB, GGT] shared sections

    # layout: gather group -> span (padded to x128) -> tile sections.
    # Each 128-edge block gets one drel column per covered tile PAIR: edges
    # of the pair's second tile store dstrel+128 and the one-hot is built
    # with a single 256-wide is_equal against iota256.
    ginfos = []
    tot_blocks = 0
    tot_cols = 0
    for g in range(NGG):
        gi = {"blk0": tot_blocks, "col0": tot_cols, "spans": {}, "tb": {},
              "bcols": [], "tmm": {}}
        assert KB == 1
        b = 0
        span_len = int(sec_len[g, b].sum())
        kgb = -(-span_len // P)
        gi["spans"][b] = (0, kgb)
        so = 0
        sec_rng = []                    # (t, start, end) in span coords
        for t in range(GGT):
            stb = int(sec_len[g, b, t])
            if stb == 0:
                continue
            gi["tb"][(b, t)] = (0, 0, 0, so)
            sec_rng.append((t, so, so + stb))
            so += stb
        # per block: covered tiles -> pair columns
        gcol = 0
        bcols = []                      # per block: (colbase, t0, ntiles)
        for j in range(kgb):
            lo, hi = j * P, (j + 1) * P
            tl_cov = [t for (t, a, e) in sec_rng if a < hi and e > lo]
            if not tl_cov:
                bcols.append((gcol, -1, 0))
                continue
            t0 = tl_cov[0]
            nt = len(tl_cov)
            assert tl_cov == list(range(t0, t0 + nt))
            bcols.append((gcol, t0, nt))
            gcol += -(-nt // 2)
        gi["bcols"] = bcols
        # per tile: ordered (block, pair m, side) matmul list
        for (t, a, e) in sec_rng:
            lst = []
            for j in range(a // P, (e - 1) // P + 1):
                colbase, t0, nt = bcols[j]
                if t0 < 0:
                    continue
                m = (t - t0) // 2
                side = (t - t0) % 2
                lst.append((j, m, side))
            gi["tmm"][t] = lst
        gi["blocks"] = kgb
        gi["ncols"] = gcol
        tot_blocks += kgb
        tot_cols += gcol
        ginfos.append(gi)
    tot_idx = tot_blocks * P

    # per-core run starts in the sorted edge list ((c, g, b, t)-major order)
    flat = counts.reshape(-1)
    starts = np.zeros(flat.size, dtype=np.int64)
    starts[1:] = np.cumsum(flat)[:-1]
    starts = starts.reshape(CORES, NGG, KB, GGT)

    per_core = []
    for c in range(CORES):
        lidx_flat = np.zeros(tot_idx, dtype=np.int16)
        drel = np.full((P, tot_cols), -1.0, dtype=np.float32)
        for g in range(NGG):
            gi = ginfos[g]
            bcol_base = np.array([bc[0] for bc in gi["bcols"]], dtype=np.int64)
            bcol_t0 = np.array([bc[1] for bc in gi["bcols"]], dtype=np.int64)
            for b, (bo, kgb) in gi["spans"].items():
                span_i0 = (gi["blk0"] + bo) * P
                for t in range(GGT):
                    if (b, t) not in gi["tb"]:
                        continue
                    so = gi["tb"][(b, t)][3]
                    n = int(counts[c, g, b, t])
                    if n == 0:
                        continue
                    s0 = int(starts[c, g, b, t])
                    pos0 = span_i0 + so
                    lidx_flat[pos0:pos0 + n] = s_lrow[s0:s0 + n]
                    q = so + np.arange(n)
                    j = q // P
                    pp = (pos0 + np.arange(n)) % P
                    toff = t - bcol_t0[j]
                    cols = gi["col0"] + bcol_base[j] + toff // 2
                    drel[pp, cols] = s_drel[s0:s0 + n] + 128.0 * (toff % 2)
        idx16 = np.tile(lidx_flat.reshape(tot_idx // 16, 16).T, (8, 1)).copy()
        per_core.append({"idx16": idx16, "dstrel": drel})

    meta = {
        "ginfos": ginfos,
        "tot_idx": tot_idx,
        "tot_cols": tot_cols,
        "dinv": dinv,
    }
    return meta, per_core


def _build_program(meta, cfg):
    ginfos = meta["ginfos"]
    tot_idx = meta["tot_idx"]
    tot_cols = meta["tot_cols"]
    CORES, TILES, RPC = cfg.CORES, cfg.TILES, cfg.RPC
    NGG, KB, GGT, GT = cfg.NGG, cfg.KB, cfg.GGT, cfg.GT
    HALF, HTILES, WPG, GR = cfg.HALF, cfg.HTILES, cfg.WPG, cfg.GR
    HT2 = TILES // 2

    kmax_g = max(gi["blocks"] for gi in ginfos)

    nc = bacc.Bacc("TRN2", debug=False)
    nc.num_devices = CORES

    xT_in = nc.dram_tensor("xT", [D, RPC], _BF16, kind="ExternalInput")
    dinv1_in = nc.dram_tensor("dinv1", [P, TILES], _F32, kind="ExternalInput")
    dinv2_in = nc.dram_tensor("dinv2", [P, TILES], _F32, kind="ExternalInput")
    w_in = [nc.dram_tensor(f"W{i + 1}", [D, D], _F32, kind="ExternalInput")
            for i in range(3)]
    bias_in = [nc.dram_tensor(f"bias{i + 1}", [P, D], _F32,
                              kind="ExternalInput") for i in range(3)]
    iota_in = nc.dram_tensor("iota", [P, 2 * P], _BF16,
                            kind="ExternalInput")
    idx_in = nc.dram_tensor("idx16", [P, tot_idx // 16], _I16,
                            kind="ExternalInput")
    drel_in = nc.dram_tensor("dstrel", [P, tot_cols], _F32,
                             kind="ExternalInput")
    out_t = nc.dram_tensor("out", [RPC, D], _F32, kind="ExternalOutput")

    with tile.TileContext(nc) as tc:
        with tc.tile_pool(name="dram", bufs=1, space="DRAM") as dram, \
             tc.tile_pool(name="const", bufs=1) as cst, \
             tc.tile_pool(name="msgp", bufs=cfg.MSGBUFS) as msgp, \
             tc.tile_pool(name="qp", bufs=cfg.QBUFS) as qp, \
             tc.tile_pool(name="qpl", bufs=24) as qpl, \
             tc.tile_pool(name="gsp", bufs=2) as gsp, \
             tc.tile_pool(name="psp", bufs=3) as psp, \
             tc.tile_pool(name="rsp", bufs=2) as rsp, \
             tc.tile_pool(name="hsp", bufs=2) as hsp, \
             tc.tile_pool(name="wk", bufs=4) as wk, \
             tc.tile_pool(name="htp", bufs=4) as htpp, \
             tc.tile_pool(name="psa", bufs=cfg.PSABUFS, space="PSUM") as psa, \
             tc.tile_pool(name="psg", bufs=2, space="PSUM") as psg, \
             tc.tile_pool(name="pst", bufs=2, space="PSUM") as pst:

            nc.gpsimd.load_library(library_config.mlp)

            gtabs = [[dram.tile([HALF, 2 * D], _BF16, name=f"gt{L}_{b}")
                      for b in range(KB)] for L in range(cfg.LAYERS)]
            partials = [[dram.tile([GR // 2, D], _BF16, name=f"part{L}_{h}")
                         for h in range(2)] for L in range(cfg.LAYERS)]
            rsouts = [[dram.tile([RPC // 2, D], _BF16,
                                 name=f"rsout{L}_{h}") for h in range(2)]
                      for L in range(cfg.LAYERS)]

            iota = cst.tile([P, 2 * P], _BF16)
            nc.sync.dma_start(iota[:], iota_in[:])
            ident = cst.tile([P, P], _F32)
            make_identity(nc, ident[:])
            dinv1 = cst.tile([P, TILES], _F32)
            nc.sync.dma_start(dinv1[:], dinv1_in[:])
            dinv2 = cst.tile([P, TILES], _F32)
            nc.sync.dma_start(dinv2[:], dinv2_in[:])
            g2b = cst.tile([P, TILES * D], _BF16)
            ws, bs = [], []
            for i in range(3):
                w = cst.tile([D, D], _F32, name=f"w{i}")
                nc.sync.dma_start(w[:], w_in[i][:])
                wb = cst.tile([D, D], _BF16, name=f"wb{i}")
                nc.vector.tensor_copy(wb[:], w[:])
                ws.append(wb)
                bt = cst.tile([P, D], _F32, name=f"b{i}")
                nc.sync.dma_start(bt[:], bias_in[i][:])
                bs.append(bt)
            ht0 = cst.tile([D, RPC], _BF16)        # layer-0 input (x.T)
            nc.sync.dma_start(ht0[:, :RPC // 2], xT_in[:, :RPC // 2])
            nc.sync.dma_start(ht0[:, RPC // 2:], xT_in[:, RPC // 2:])



            idx_sb = cst.tile([P, tot_idx // 16], _I16)
            idx_cut = (ginfos[4]["blk0"] * P) // 16 if NGG > 4 else 0
            if idx_cut:
                nc.sync.dma_start(idx_sb[:, :idx_cut], idx_in[:, :idx_cut])
            drel_sb = cst.tile([P, tot_cols], _F32)
            nc.sync.dma_start(drel_sb[:], drel_in[:])
            if idx_cut:
                nc.sync.dma_start(idx_sb[:, idx_cut:], idx_in[:, idx_cut:])
            else:
                nc.sync.dma_start(idx_sb[:], idx_in[:])

            qctr = [0]
            cctr = [0]

            def build_q(qt, col, width):
                eng = nc.vector
                if width <= P:
                    if qctr[0] % cfg.Q_SPLIT == cfg.Q_SPLIT - 1:
                        eng = nc.gpsimd
                    qctr[0] += 1
                eng.tensor_scalar(
                    out=qt[:, :width], in0=iota[:, :width],
                    scalar1=drel_sb[:, col:col + 1], scalar2=None,
                    op0=mybir.AluOpType.is_equal)
                return eng

            def convert(dst_ap, src_ap):
                m = cctr[0] % cfg.CONV_MOD
                cctr[0] += 1
                if m == 0:
                    nc.vector.tensor_copy(dst_ap, src_ap)
                else:
                    nc.scalar.copy(dst_ap, src_ap)

            def emit_gathers(L, g):
                gi = ginfos[g]
                msg = msgp.tile([P, gi["blocks"], 2 * D], _BF16,
                                tag="msg", name=f"msg{L}_{g}",
                                padded_shape=[P, kmax_g, 2 * D])
                for b, (bo, kgb) in gi["spans"].items():
                    i0 = (gi["blk0"] + bo) * P
                    nidx = kgb * P
                    nc.gpsimd.dma_gather(
                        msg[:, bo:bo + kgb, :],
                        gtabs[L][b][:],
                        idx_sb[:, i0 // 16:(i0 + nidx) // 16],
                        nidx, nidx, 2 * D,
                        single_packet=False)
                return msg

            def emit_process(L, g, msg):
                gi = ginfos[g]
                bcols = gi["bcols"]
                qtiles = {}                      # (j, m) -> q tile

                def get_q(j, m):
                    key = (j, m)
                    if key not in qtiles:
                        colbase, t0, nt = bcols[j]
                        width = 2 * P if (nt - 2 * m) >= 2 else P
                        if (width <= P
                                and qctr[0] % cfg.Q_SPLIT == cfg.Q_SPLIT - 1):
                            qt = qpl.tile([P, P], _BF16, tag="qpl",
                                          name=f"q{L}_{g}_{j}_{m}")
                        else:
                            qt = qp.tile([P, 2 * P], _BF16, tag="q",
                                         name=f"q{L}_{g}_{j}_{m}")
                        build_q(qt, gi["col0"] + colbase + m, width)
                        qtiles[key] = qt
                    return qtiles[key]

                for wg in range(WPG):
                    pstage = psp.tile([P, GT * D], _BF16, tag="ps",
                                      name=f"ps{L}_{g}_{wg}")
                    pa = psa.tile([P, GT * D], _F32, tag="pa",
                                  name=f"pa{L}_{g}_{wg}")
                    for tw in range(GT):
                        t = wg * GT + tw
                        tmm = gi["tmm"].get(t, [])
                        nmm = len(tmm)
                        assert nmm > 0
                        for done, (j, m, side) in enumerate(tmm):
                            qt = get_q(j, m)
                            nc.tensor.matmul(
                                pa[:, tw * D:(tw + 1) * D],
                                lhsT=qt[:, side * P:(side + 1) * P],
                                rhs=msg[:, j, :D],
                                start=(done == 0),
                                stop=(done == nmm - 1))
                    convert(pstage[:], pa[:])
                    gr = g * GGT + wg * GT          # global tile index
                    half = gr // (CORES * HT2)
                    r0 = (gr - half * CORES * HT2) * P
                    dst = partials[L][half][r0:r0 + GT * P, :].rearrange(
                        "(j p) c -> p j c", j=GT, p=P)
                    nc.sync.dma_start(dst, pstage[:].rearrange(
                        "p (j c) -> p j c", j=GT, c=D))

            def emit_phase_a0():
                """Layer-0 phase A from ht0 (no epilogue)."""
                for half in range(2):
                    gstage = gsp.tile([P, HT2 * 2 * D], _BF16, tag="gs",
                                      name=f"gs0_{half}")
                    nc.vector.memset(gstage[:], 0.0)
                    for th in range(HT2):
                        t = half * HT2 + th
                        pg = psg.tile([P, D], _F32, tag="pg",
                                      name=f"pg0_{t}")
                        nc.tensor.matmul(pg[:],
                                         lhsT=ht0[:, t * P:(t + 1) * P],
                                         rhs=ws[0][:], start=True, stop=True)
                        nc.scalar.mul(
                            gstage[:, th * 2 * D:th * 2 * D + D],
                            pg[:], dinv1[:, t:t + 1])
                        g2 = wk.tile([P, D], _F32, tag="g2",
                                     name=f"g2_0_{t}")
                        nc.scalar.mul(g2[:], pg[:], dinv2[:, t:t + 1])
                        nc.vector.tensor_tensor(
                            out=g2b[:, t * D:(t + 1) * D], in0=g2[:],
                            in1=bs[0][:], op=mybir.AluOpType.add)
                    r0 = half * HT2 * P
                    dst = gtabs[0][0][r0:r0 + HT2 * P, :].rearrange(
                        "(j p) c -> p j c", j=HT2, p=P)
                    nc.sync.dma_start(dst, gstage[:].rearrange(
                        "p (j c) -> p j c", j=HT2, c=2 * D))

            def make_half_closures(L, half):
                """Closures: epilogue of layer L-1 results for own tiles of
                `half`, fused with phase A of layer L (or the final output
                when L == LAYERS).  Call in order; interleavable."""
                final = (L == cfg.LAYERS)
                state = {}

                def load_rs():
                    rsall = rsp.tile([P, HT2 * D], _BF16, tag="rs",
                                     name=f"rsall{L}_{half}")
                    src = rsouts[L - 1][half][:].rearrange(
                        "(j p) c -> p j c", j=HT2, p=P)
                    nc.scalar.dma_start(rsall[:].rearrange(
                        "p (j c) -> p j c", j=HT2, c=D), src)
                    state["rs"] = rsall
                    if final:
                        state["hs"] = hsp.tile([P, HT2 * D], _F32, tag="hs",
                                               name=f"hs_{half}")
                    else:
                        gst = gsp.tile([P, HT2 * 2 * D], _BF16, tag="gs",
                                       name=f"gs{L}_{half}")
                        state["gs"] = gst

                def tile_work(th):
                    t = half * HT2 + th
                    rsall = state["rs"]
                    v = wk.tile([P, D], _BF16, tag="v", name=f"v{L}_{t}")
                    nc.scalar.mul(v[:], rsall[:, th * D:(th + 1) * D],
                                  dinv1[:, t:t + 1])
                    nc.vector.tensor_tensor(
                        out=v[:], in0=v[:],
                        in1=g2b[:, t * D:(t + 1) * D],
                        op=mybir.AluOpType.add)
                    if final:
                        nc.scalar.activation(
                            state["hs"][:, th * D:(th + 1) * D], v[:],
                            mybir.ActivationFunctionType.Lrelu,
                            bias=0.0, scale=1.0, alpha=NEG_SLOPE)
                        return
                    h = wk.tile([P, D], _F32, tag="h", name=f"h{L}_{t}")
                    nc.scalar.activation(
                        h[:], v[:], mybir.ActivationFunctionType.Lrelu,
                        bias=0.0, scale=1.0, alpha=NEG_SLOPE)
                    pt = pst.tile([D, P], _F32, tag="pt", name=f"pt{L}_{t}")
                    nc.tensor.transpose(pt[:], h[:], ident[:])
                    htp = htpp.tile([D, P], _BF16, tag="ht",
                                    name=f"htp{L}_{t}")
                    if half == 0:
                        nc.scalar.copy(htp[:], pt[:])
                    else:
                        nc.vector.tensor_copy(htp[:], pt[:])
                    pg = psg.tile([P, D], _F32, tag="pg", name=f"pg{L}_{t}")
                    nc.tensor.matmul(pg[:], lhsT=htp[:], rhs=ws[L][:],
                                     start=True, stop=True)
                    gslice = state["gs"][:, th * 2 * D:th * 2 * D + D]
                    if half == 0:
                        nc.scalar.mul(gslice, pg[:], dinv1[:, t:t + 1])
                    else:
                        nc.vector.tensor_scalar_mul(gslice, pg[:],
                                                    dinv1[:, t:t + 1])
                    g2 = wk.tile([P, D], _F32, tag="g2", name=f"g2_{L}_{t}")
                    nc.scalar.mul(g2[:], pg[:], dinv2[:, t:t + 1])
                    nc.vector.tensor_tensor(
                        out=g2b[:, t * D:(t + 1) * D], in0=g2[:],
                        in1=bs[L][:], op=mybir.AluOpType.add)

                def flush():
                    r0 = half * HT2 * P
                    if final:
                        dst = out_t[r0:r0 + HT2 * P, :].rearrange(
                            "(j p) c -> p j c", j=HT2, p=P)
                        nc.sync.dma_start(dst, state["hs"][:].rearrange(
                            "p (j c) -> p j c", j=HT2, c=D))
                    else:
                        dst = gtabs[L][0][r0:r0 + HT2 * P, :].rearrange(
                            "(j p) c -> p j c", j=HT2, p=P)
                        nc.sync.dma_start(dst, state["gs"][:].rearrange(
                            "p (j c) -> p j c", j=HT2, c=2 * D))

                return ([load_rs] + [lambda th=th: tile_work(th)
                                     for th in range(HT2)] + [flush])

            def emit_rs(L, half):
                nc.gpsimd.collective_compute(
                    "ReduceScatter",
                    mybir.AluOpType.add,
                    replica_groups=[list(range(CORES))],
                    ins=[partials[L][half][:]],
                    outs=[rsouts[L][half][:]],
                )

            NHALF = NGG // 2
            ILV_START = NHALF + 5       # first group that interleaves epi

            for L in range(cfg.LAYERS):
                if L == 0:
                    emit_phase_a0()
                msgs = {}
                for gg in range(min(3, NGG)):
                    msgs[gg] = emit_gathers(L, gg)
                pend = []
                for g in range(NGG):
                    if g + 3 < NGG:
                        msgs[g + 3] = emit_gathers(L, g + 3)
                    emit_process(L, g, msgs.pop(g))
                    if g == NHALF + 2:
                        emit_rs(L, 0)
                        pend = make_half_closures(L + 1, 0)
                    if g >= ILV_START and pend:
                        take = -(-len(pend) // (NGG - g))
                        for fn in pend[:take]:
                            fn()
                        pend = pend[take:]
                for fn in pend:
                    fn()
                emit_rs(L, 1)
                for fn in make_half_closures(L + 1, 1):
                    fn()

    nc.compile()
    return nc


def make_in_maps(x, Ws, bss, meta, per_core, cfg):
    dinv = meta["dinv"]
    CORES, NPC, RPC, TILES = cfg.CORES, cfg.NPC, cfg.RPC, cfg.TILES
    import ml_dtypes
    iota_np = np.broadcast_to(np.arange(2 * P).astype(ml_dtypes.bfloat16),
                              (P, 2 * P)).copy()
    in_maps = []
    for c in range(CORES):
        sl = slice(c * NPC, (c + 1) * NPC)
        xT = np.zeros((D, RPC), np.float32)
        xT[:, :NPC] = x[sl].T
        d1c = np.zeros(RPC, np.float32)
        d1c[:NPC] = dinv[sl]
        d1 = d1c.reshape(TILES, P).T.copy()
        d2 = (d1 * d1).astype(np.float32)
        im = {
            "xT": xT.astype(ml_dtypes.bfloat16),
            "dinv1": d1,
            "dinv2": d2,
            "iota": iota_np,
            "idx16": per_core[c]["idx16"],
            "dstrel": per_core[c]["dstrel"],
        }
        for i in range(3):
            im[f"W{i + 1}"] = Ws[i]
            im[f"bias{i + 1}"] = np.broadcast_to(
                bss[i], (P, D)).astype(np.float32).copy()
        in_maps.append(im)
    return in_maps


_CACHE = {}


def kernel(x, edge_index, W1, b1, W2, b2, W3, b3):
    cfg = DEFAULT_CFG
    x = np.asarray(x, dtype=np.float32)
    Ws = [np.asarray(w, dtype=np.float32) for w in (W1, W2, W3)]
    bss = [np.asarray(b, dtype=np.float32) for b in (b1, b2, b3)]

    ei = np.asarray(edge_index)
    key = hash(ei[:, ::997].tobytes()) ^ hash(ei.shape)
    if key not in _CACHE:
        meta, per_core = _preprocess(ei, cfg)
        nc = _build_program(meta, cfg)
        _CACHE[key] = (meta, per_core, nc)
    meta, per_core, nc = _CACHE[key]

    in_maps = make_in_maps(x, Ws, bss, meta, per_core, cfg)
    res = run_bass_kernel_spmd(nc, in_maps, core_ids=list(range(cfg.CORES)))
    out = np.empty((cfg.N, D), np.float32)
    for c in range(cfg.CORES):
        out[c * cfg.NPC:(c + 1) * cfg.NPC] = res.results[c]["out"][:cfg.NPC]
    return out


# revision 26
# speedup vs baseline: 1.0065x; 1.0065x over previous
"""3-layer GCN (GCNConv x3 + LeakyReLU, PyG semantics) on 8 Trainium2 cores.

Strategy (source-partitioned, ReduceScatter):
  - Core c owns nodes [c*NPC, (c+1)*NPC) and processes the edges whose SOURCE
    it owns.  Self loops are added as explicit edges, so a whole layer is
        OUT[d] = lrelu(dinv[d] * sum_{e: dst=d} G[src_e] + bias),
    with G = dinv * (H @ W) computed locally per core (no feature exchange).
  - Per layer: phase A computes G for own nodes into two local DRAM gather
    tables (low/high half of own rows, int16-indexable, 256B rows), then
    dma_gather over globally dst-sorted edges + one-hot matmul segment-sum
    into PSUM per global dst tile (784 tiles across all cores), converted to
    bf16 and written into a row-major [100352, 64] partial-sum table.
  - One ReduceScatter(add) delivers each core the full sum for its own
    12544 rows -- an output-sized collective (~57us) instead of AllGathering
    the whole feature table (~284us).
  - Epilogue per own tile: scale by dinv (ACT), add bias (DVE), LeakyReLU
    (ACT), PE transpose, and the next layer's phase-A matmul (bf16).
  - One-hot q matrices (is_equal(iota, drel)) are split across DVE and Pool;
    PSUM->bf16 converts across ACT/DVE, keeping every engine below the DMA
    roofline.  Gathers use one large call per (gather-group, table-half)
    span to amortize the ~1us SWDGE fixed cost per call.

The Bass program is SPMD: one program, per-core data.  Section lengths are
shared across cores (max over cores, padded with dummy edges whose one-hot
column is zero: dstrel = -1).
"""
import sys

sys.path.insert(0, "/opt/trn_rl_repo")

import numpy as np

import concourse.bacc as bacc
import concourse.mybir as mybir
import concourse.tile as tile
from concourse import library_config
from concourse.bass_utils import run_bass_kernel_spmd
from concourse.masks import make_identity

_F32 = mybir.dt.float32
_BF16 = mybir.dt.bfloat16
_I16 = mybir.dt.int16
P = 128
D = 64
NEG_SLOPE = 0.01


class Cfg:
    def __init__(self, n_nodes=100000, cores=8):
        self.N = n_nodes
        self.CORES = cores
        self.NPC = self.N // cores            # nodes owned per core
        self.TILES = (self.NPC + P - 1) // P  # own-node tiles per core (98)
        self.RPC = self.TILES * P             # padded rows per core (12544)
        self.GR = cores * self.RPC            # global padded rows (100352)
        self.GTILES = cores * self.TILES      # global dst tiles (784)
        self.KB = 1                           # source buckets (gather tables)
        assert self.RPC % self.KB == 0
        self.HALF = self.RPC // self.KB       # rows per gather table (12544)
        assert self.HALF <= 32767
        self.GGT = 28                         # dst tiles per gather group
        assert self.GTILES % self.GGT == 0
        self.NGG = self.GTILES // self.GGT    # gather groups (28)
        self.GT = 7                           # dst tiles per write group
        assert self.GGT % self.GT == 0
        self.WPG = self.GGT // self.GT        # write groups per gather group
        self.HTILES = self.TILES // self.KB   # own tiles per table (98)
        assert self.TILES % self.KB == 0
        self.LAYERS = 3
        self.MSGBUFS = 3
        self.QBUFS = 12
        self.PSABUFS = 3
        # engine split knobs
        self.Q_SPLIT = 1000000      # every Q_SPLIT-th q op goes to Pool
        self.CONV_MOD = 1000000     # converts: i%MOD==0 -> DVE, else ACT


DEFAULT_CFG = Cfg()


def _preprocess(edge_index, cfg):
    """Sort/pack edges; build per-core device arrays and shared metadata."""
    src0 = np.asarray(edge_index[0], dtype=np.int64)
    dst0 = np.asarray(edge_index[1], dtype=np.int64)
    N, CORES, NPC, TILES = cfg.N, cfg.CORES, cfg.NPC, cfg.TILES
    NGG, KB, GGT, HALF = cfg.NGG, cfg.KB, cfg.GGT, cfg.HALF

    src = src0
    dst = dst0

    deg = np.bincount(dst, minlength=N).astype(np.float32) + 1.0
    dinv = (1.0 / np.sqrt(deg)).astype(np.float32)

    owner = src // NPC                      # processing core (source owner)
    slocal = src % NPC
    bucket = slocal // HALF                 # gather-table half
    lrow = (slocal % HALF).astype(np.int16)

    downer = dst // NPC
    dloc = dst % NPC
    otile = dloc // P                       # owner-local tile 0..97
    HT2 = TILES // 2
    # (half, owner, tile-in-half): ReduceScatter halves then split per owner
    gtile = (otile // HT2) * (CORES * HT2) + downer * HT2 + otile % HT2
    drel_v = (dloc % P).astype(np.float32)
    grp = gtile // GGT
    tl = gtile % GGT

    counts = np.zeros((CORES, NGG, KB, GGT), dtype=np.int64)
    np.add.at(counts, (owner, grp, bucket, tl), 1)
    order = np.lexsort((tl, bucket, grp, owner))
    s_lrow = lrow[order]
    s_drel = drel_v[order]

    sec_len = counts.max(axis=0)            # [NGG, KB, GGT] shared sections

    # layout: gather group -> span (padded to x128) -> tile sections.
    # Each 128-edge block gets one drel column per covered tile PAIR: edges
    # of the pair's second tile store dstrel+128 and the one-hot is built
    # with a single 256-wide is_equal against iota256.
    ginfos = []
    tot_blocks = 0
    tot_cols = 0
    for g in range(NGG):
        gi = {"blk0": tot_blocks, "col0": tot_cols, "spans": {}, "tb": {},
              "bcols": [], "tmm": {}}
        assert KB == 1
        b = 0
        span_len = int(sec_len[g, b].sum())
        kgb = -(-span_len // P)
        gi["spans"][b] = (0, kgb)
        so = 0
        sec_rng = []                    # (t, start, end) in span coords
        for t in range(GGT):
            stb = int(sec_len[g, b, t])
            if stb == 0:
                continue
            gi["tb"][(b, t)] = (0, 0, 0, so)
            sec_rng.append((t, so, so + stb))
            so += stb
        # per block: covered tiles -> pair columns
        gcol = 0
        bcols = []                      # per block: (colbase, t0, ntiles)
        for j in range(kgb):
            lo, hi = j * P, (j + 1) * P
            tl_cov = [t for (t, a, e) in sec_rng if a < hi and e > lo]
            if not tl_cov:
                bcols.append((gcol, -1, 0))
                continue
            t0 = tl_cov[0]
            nt = len(tl_cov)
            assert tl_cov == list(range(t0, t0 + nt))
            bcols.append((gcol, t0, nt))
            gcol += -(-nt // 2)
        gi["bcols"] = bcols
        # per tile: ordered (block, pair m, side) matmul list
        for (t, a, e) in sec_rng:
            lst = []
            for j in range(a // P, (e - 1) // P + 1):
                colbase, t0, nt = bcols[j]
                if t0 < 0:
                    continue
                m = (t - t0) // 2
                side = (t - t0) % 2
                lst.append((j, m, side))
            gi["tmm"][t] = lst
        gi["blocks"] = kgb
        gi["ncols"] = gcol
        tot_blocks += kgb
        tot_cols += gcol
        ginfos.append(gi)
    tot_idx = tot_blocks * P

    # per-core run starts in the sorted edge list ((c, g, b, t)-major order)
    flat = counts.reshape(-1)
    starts = np.zeros(flat.size, dtype=np.int64)
    starts[1:] = np.cumsum(flat)[:-1]
    starts = starts.reshape(CORES, NGG, KB, GGT)

    per_core = []
    for c in range(CORES):
        lidx_flat = np.zeros(tot_idx, dtype=np.int16)
        drel = np.full((P, tot_cols), -1.0, dtype=np.float32)
        for g in range(NGG):
            gi = ginfos[g]
            bcol_base = np.array([bc[0] for bc in gi["bcols"]], dtype=np.int64)
            bcol_t0 = np.array([bc[1] for bc in gi["bcols"]], dtype=np.int64)
            for b, (bo, kgb) in gi["spans"].items():
                span_i0 = (gi["blk0"] + bo) * P
                for t in range(GGT):
                    if (b, t) not in gi["tb"]:
                        continue
                    so = gi["tb"][(b, t)][3]
                    n = int(counts[c, g, b, t])
                    if n == 0:
                        continue
                    s0 = int(starts[c, g, b, t])
                    pos0 = span_i0 + so
                    lidx_flat[pos0:pos0 + n] = s_lrow[s0:s0 + n]
                    q = so + np.arange(n)
                    j = q // P
                    pp = (pos0 + np.arange(n)) % P
                    toff = t - bcol_t0[j]
                    cols = gi["col0"] + bcol_base[j] + toff // 2
                    drel[pp, cols] = s_drel[s0:s0 + n] + 128.0 * (toff % 2)
        idx16 = np.tile(lidx_flat.reshape(tot_idx // 16, 16).T, (8, 1)).copy()
        per_core.append({"idx16": idx16, "dstrel": drel})

    meta = {
        "ginfos": ginfos,
        "tot_idx": tot_idx,
        "tot_cols": tot_cols,
        "dinv": dinv,
    }
    return meta, per_core


def _build_program(meta, cfg):
    ginfos = meta["ginfos"]
    tot_idx = meta["tot_idx"]
    tot_cols = meta["tot_cols"]
    CORES, TILES, RPC = cfg.CORES, cfg.TILES, cfg.RPC
    NGG, KB, GGT, GT = cfg.NGG, cfg.KB, cfg.GGT, cfg.GT
    HALF, HTILES, WPG, GR = cfg.HALF, cfg.HTILES, cfg.WPG, cfg.GR
    HT2 = TILES // 2

    kmax_g = max(gi["blocks"] for gi in ginfos)

    nc = bacc.Bacc("TRN2", debug=False)
    nc.num_devices = CORES

    xT_in = nc.dram_tensor("xT", [D, RPC], _BF16, kind="ExternalInput")
    dinv1_in = nc.dram_tensor("dinv1", [P, TILES], _F32, kind="ExternalInput")
    dinv2_in = nc.dram_tensor("dinv2", [P, TILES], _F32, kind="ExternalInput")
    w_in = [nc.dram_tensor(f"W{i + 1}", [D, D], _F32, kind="ExternalInput")
            for i in range(3)]
    bias_in = [nc.dram_tensor(f"bias{i + 1}", [P, D], _F32,
                              kind="ExternalInput") for i in range(3)]
    iota_in = nc.dram_tensor("iota", [P, 2 * P], _BF16,
                            kind="ExternalInput")
    idx_in = nc.dram_tensor("idx16", [P, tot_idx // 16], _I16,
                            kind="ExternalInput")
    drel_in = nc.dram_tensor("dstrel", [P, tot_cols], _F32,
                             kind="ExternalInput")
    out_t = nc.dram_tensor("out", [RPC, D], _F32, kind="ExternalOutput")

    with tile.TileContext(nc) as tc:
        with tc.tile_pool(name="dram", bufs=1, space="DRAM") as dram, \
             tc.tile_pool(name="const", bufs=1) as cst, \
             tc.tile_pool(name="msgp", bufs=cfg.MSGBUFS) as msgp, \
             tc.tile_pool(name="qp", bufs=cfg.QBUFS) as qp, \
             tc.tile_pool(name="qpl", bufs=24) as qpl, \
             tc.tile_pool(name="gsp", bufs=2) as gsp, \
             tc.tile_pool(name="psp", bufs=3) as psp, \
             tc.tile_pool(name="rsp", bufs=2) as rsp, \
             tc.tile_pool(name="hsp", bufs=2) as hsp, \
             tc.tile_pool(name="wk", bufs=4) as wk, \
             tc.tile_pool(name="htp", bufs=4) as htpp, \
             tc.tile_pool(name="psa", bufs=cfg.PSABUFS, space="PSUM") as psa, \
             tc.tile_pool(name="psg", bufs=2, space="PSUM") as psg, \
             tc.tile_pool(name="pst", bufs=2, space="PSUM") as pst:

            nc.gpsimd.load_library(library_config.mlp)

            gtabs = [[dram.tile([HALF, 2 * D], _BF16, name=f"gt{L}_{b}")
                      for b in range(KB)] for L in range(cfg.LAYERS)]
            partials = [[dram.tile([GR // 2, D], _BF16, name=f"part{L}_{h}")
                         for h in range(2)] for L in range(cfg.LAYERS)]
            rsouts = [[dram.tile([RPC // 2, D], _BF16,
                                 name=f"rsout{L}_{h}") for h in range(2)]
                      for L in range(cfg.LAYERS)]

            iota = cst.tile([P, 2 * P], _BF16)
            nc.sync.dma_start(iota[:], iota_in[:])
            ident = cst.tile([P, P], _F32)
            make_identity(nc, ident[:])
            dinv1 = cst.tile([P, TILES], _F32)
            nc.sync.dma_start(dinv1[:], dinv1_in[:])
            dinv2 = cst.tile([P, TILES], _F32)
            nc.sync.dma_start(dinv2[:], dinv2_in[:])
            g2b = cst.tile([P, TILES * D], _BF16)
            ws, bs = [], []
            for i in range(3):
                w = cst.tile([D, D], _F32, name=f"w{i}")
                nc.sync.dma_start(w[:], w_in[i][:])
                wb = cst.tile([D, D], _BF16, name=f"wb{i}")
                nc.vector.tensor_copy(wb[:], w[:])
                ws.append(wb)
                bt = cst.tile([P, D], _F32, name=f"b{i}")
                nc.sync.dma_start(bt[:], bias_in[i][:])
                bs.append(bt)
            ht0 = cst.tile([D, RPC], _BF16)        # layer-0 input (x.T)
            nc.sync.dma_start(ht0[:, :RPC // 2], xT_in[:, :RPC // 2])
            nc.sync.dma_start(ht0[:, RPC // 2:], xT_in[:, RPC // 2:])



            idx_sb = cst.tile([P, tot_idx // 16], _I16)
            idx_cut = (ginfos[4]["blk0"] * P) // 16 if NGG > 4 else 0
            if idx_cut:
                nc.sync.dma_start(idx_sb[:, :idx_cut], idx_in[:, :idx_cut])
            drel_sb = cst.tile([P, tot_cols], _F32)
            nc.sync.dma_start(drel_sb[:], drel_in[:])
            if idx_cut:
                nc.sync.dma_start(idx_sb[:, idx_cut:], idx_in[:, idx_cut:])
            else:
                nc.sync.dma_start(idx_sb[:], idx_in[:])

            qctr = [0]
            cctr = [0]

            def build_q(qt, col, width):
                eng = nc.vector
                if width <= P:
                    if qctr[0] % cfg.Q_SPLIT == cfg.Q_SPLIT - 1:
                        eng = nc.gpsimd
                    qctr[0] += 1
                eng.tensor_scalar(
                    out=qt[:, :width], in0=iota[:, :width],
                    scalar1=drel_sb[:, col:col + 1], scalar2=None,
                    op0=mybir.AluOpType.is_equal)
                return eng

            def convert(dst_ap, src_ap):
                m = cctr[0] % cfg.CONV_MOD
                cctr[0] += 1
                if m == 0:
                    nc.vector.tensor_copy(dst_ap, src_ap)
                else:
                    nc.scalar.copy(dst_ap, src_ap)

            def emit_gathers(L, g):
                gi = ginfos[g]
                msg = msgp.tile([P, gi["blocks"], 2 * D], _BF16,
                                tag="msg", name=f"msg{L}_{g}",
                                padded_shape=[P, kmax_g, 2 * D])
                for b, (bo, kgb) in gi["spans"].items():
                    i0 = (gi["blk0"] + bo) * P
                    nidx = kgb * P
                    nc.gpsimd.dma_gather(
                        msg[:, bo:bo + kgb, :],
                        gtabs[L][b][:],
                        idx_sb[:, i0 // 16:(i0 + nidx) // 16],
                        nidx, nidx, 2 * D,
                        single_packet=False)
                return msg

            def emit_process(L, g, msg):
                gi = ginfos[g]
                bcols = gi["bcols"]
                qtiles = {}                      # (j, m) -> q tile

                def get_q(j, m):
                    key = (j, m)
                    if key not in qtiles:
                        colbase, t0, nt = bcols[j]
                        width = 2 * P if (nt - 2 * m) >= 2 else P
                        if (width <= P
                                and qctr[0] % cfg.Q_SPLIT == cfg.Q_SPLIT - 1):
                            qt = qpl.tile([P, P], _BF16, tag="qpl",
                                          name=f"q{L}_{g}_{j}_{m}")
                        else:
                            qt = qp.tile([P, 2 * P], _BF16, tag="q",
                                         name=f"q{L}_{g}_{j}_{m}")
                        build_q(qt, gi["col0"] + colbase + m, width)
                        qtiles[key] = qt
                    return qtiles[key]

                for wg in range(WPG):
                    pstage = psp.tile([P, GT * D], _BF16, tag="ps",
                                      name=f"ps{L}_{g}_{wg}")
                    pa = psa.tile([P, GT * D], _F32, tag="pa",
                                  name=f"pa{L}_{g}_{wg}")
                    for tw in range(GT):
                        t = wg * GT + tw
                        tmm = gi["tmm"].get(t, [])
                        nmm = len(tmm)
                        assert nmm > 0
                        for done, (j, m, side) in enumerate(tmm):
                            qt = get_q(j, m)
                            nc.tensor.matmul(
                                pa[:, tw * D:(tw + 1) * D],
                                lhsT=qt[:, side * P:(side + 1) * P],
                                rhs=msg[:, j, :D],
                                start=(done == 0),
                                stop=(done == nmm - 1))
                    convert(pstage[:], pa[:])
                    gr = g * GGT + wg * GT          # global tile index
                    half = gr // (CORES * HT2)
                    r0 = (gr - half * CORES * HT2) * P
                    dst = partials[L][half][r0:r0 + GT * P, :].rearrange(
                        "(j p) c -> p j c", j=GT, p=P)
                    nc.sync.dma_start(dst, pstage[:].rearrange(
                        "p (j c) -> p j c", j=GT, c=D))

            def emit_phase_a0():
                """Layer-0 phase A from ht0 (no epilogue)."""
                for half in range(2):
                    gstage = gsp.tile([P, HT2 * 2 * D], _BF16, tag="gs",
                                      name=f"gs0_{half}")
                    nc.vector.memset(gstage[:], 0.0)
                    for th in range(HT2):
                        t = half * HT2 + th
                        pg = psg.tile([P, D], _F32, tag="pg",
                                      name=f"pg0_{t}")
                        nc.tensor.matmul(pg[:],
                                         lhsT=ht0[:, t * P:(t + 1) * P],
                                         rhs=ws[0][:], start=True, stop=True)
                        nc.vector.tensor_scalar_mul(
                            gstage[:, th * 2 * D:th * 2 * D + D],
                            pg[:], dinv1[:, t:t + 1])
                        g2 = wk.tile([P, D], _F32, tag="g2",
                                     name=f"g2_0_{t}")
                        nc.scalar.mul(g2[:], pg[:], dinv2[:, t:t + 1])
                        nc.vector.tensor_tensor(
                            out=g2b[:, t * D:(t + 1) * D], in0=g2[:],
                            in1=bs[0][:], op=mybir.AluOpType.add)
                    r0 = half * HT2 * P
                    dst = gtabs[0][0][r0:r0 + HT2 * P, :].rearrange(
                        "(j p) c -> p j c", j=HT2, p=P)
                    nc.sync.dma_start(dst, gstage[:].rearrange(
                        "p (j c) -> p j c", j=HT2, c=2 * D))

            def make_half_closures(L, half):
                """Closures: epilogue of layer L-1 results for own tiles of
                `half`, fused with phase A of layer L (or the final output
                when L == LAYERS).  Call in order; interleavable."""
                final = (L == cfg.LAYERS)
                state = {}

                def load_rs():
                    rsall = rsp.tile([P, HT2 * D], _BF16, tag="rs",
                                     name=f"rsall{L}_{half}")
                    src = rsouts[L - 1][half][:].rearrange(
                        "(j p) c -> p j c", j=HT2, p=P)
                    nc.scalar.dma_start(rsall[:].rearrange(
                        "p (j c) -> p j c", j=HT2, c=D), src)
                    state["rs"] = rsall
                    if final:
                        state["hs"] = hsp.tile([P, HT2 * D], _F32, tag="hs",
                                               name=f"hs_{half}")
                    else:
                        gst = gsp.tile([P, HT2 * 2 * D], _BF16, tag="gs",
                                       name=f"gs{L}_{half}")
                        state["gs"] = gst

                def tile_work(th):
                    t = half * HT2 + th
                    rsall = state["rs"]
                    v = wk.tile([P, D], _BF16, tag="v", name=f"v{L}_{t}")
                    nc.scalar.mul(v[:], rsall[:, th * D:(th + 1) * D],
                                  dinv1[:, t:t + 1])
                    nc.vector.tensor_tensor(
                        out=v[:], in0=v[:],
                        in1=g2b[:, t * D:(t + 1) * D],
                        op=mybir.AluOpType.add)
                    if final:
                        nc.scalar.activation(
                            state["hs"][:, th * D:(th + 1) * D], v[:],
                            mybir.ActivationFunctionType.Lrelu,
                            bias=0.0, scale=1.0, alpha=NEG_SLOPE)
                        return
                    h = wk.tile([P, D], _F32, tag="h", name=f"h{L}_{t}")
                    nc.scalar.activation(
                        h[:], v[:], mybir.ActivationFunctionType.Lrelu,
                        bias=0.0, scale=1.0, alpha=NEG_SLOPE)
                    pt = pst.tile([D, P], _F32, tag="pt", name=f"pt{L}_{t}")
                    nc.tensor.transpose(pt[:], h[:], ident[:])
                    htp = htpp.tile([D, P], _BF16, tag="ht",
                                    name=f"htp{L}_{t}")
                    if half == 0:
                        nc.scalar.copy(htp[:], pt[:])
                    else:
                        nc.vector.tensor_copy(htp[:], pt[:])
                    pg = psg.tile([P, D], _F32, tag="pg", name=f"pg{L}_{t}")
                    nc.tensor.matmul(pg[:], lhsT=htp[:], rhs=ws[L][:],
                                     start=True, stop=True)
                    gslice = state["gs"][:, th * 2 * D:th * 2 * D + D]
                    if half == 0:
                        nc.scalar.mul(gslice, pg[:], dinv1[:, t:t + 1])
                    else:
                        nc.vector.tensor_scalar_mul(gslice, pg[:],
                                                    dinv1[:, t:t + 1])
                    g2 = wk.tile([P, D], _F32, tag="g2", name=f"g2_{L}_{t}")
                    nc.scalar.mul(g2[:], pg[:], dinv2[:, t:t + 1])
                    nc.vector.tensor_tensor(
                        out=g2b[:, t * D:(t + 1) * D], in0=g2[:],
                        in1=bs[L][:], op=mybir.AluOpType.add)

                def flush():
                    r0 = half * HT2 * P
                    if final:
                        dst = out_t[r0:r0 + HT2 * P, :].rearrange(
                            "(j p) c -> p j c", j=HT2, p=P)
                        nc.sync.dma_start(dst, state["hs"][:].rearrange(
                            "p (j c) -> p j c", j=HT2, c=D))
                    else:
                        dst = gtabs[L][0][r0:r0 + HT2 * P, :].rearrange(
                            "(j p) c -> p j c", j=HT2, p=P)
                        nc.sync.dma_start(dst, state["gs"][:].rearrange(
                            "p (j c) -> p j c", j=HT2, c=2 * D))

                return ([load_rs] + [lambda th=th: tile_work(th)
                                     for th in range(HT2)] + [flush])

            def emit_rs(L, half):
                nc.gpsimd.collective_compute(
                    "ReduceScatter",
                    mybir.AluOpType.add,
                    replica_groups=[list(range(CORES))],
                    ins=[partials[L][half][:]],
                    outs=[rsouts[L][half][:]],
                )

            NHALF = NGG // 2
            ILV_START = NHALF + 5       # first group that interleaves epi

            for L in range(cfg.LAYERS):
                if L == 0:
                    emit_phase_a0()
                msgs = {}
                for gg in range(min(3, NGG)):
                    msgs[gg] = emit_gathers(L, gg)
                pend = []
                for g in range(NGG):
                    if g + 3 < NGG:
                        msgs[g + 3] = emit_gathers(L, g + 3)
                    emit_process(L, g, msgs.pop(g))
                    if g == NHALF + 2:
                        emit_rs(L, 0)
                        pend = make_half_closures(L + 1, 0)
                    if g >= ILV_START and pend:
                        take = -(-len(pend) // (NGG - g))
                        for fn in pend[:take]:
                            fn()
                        pend = pend[take:]
                for fn in pend:
                    fn()
                emit_rs(L, 1)
                for fn in make_half_closures(L + 1, 1):
                    fn()

    nc.compile()
    return nc


def make_in_maps(x, Ws, bss, meta, per_core, cfg):
    dinv = meta["dinv"]
    CORES, NPC, RPC, TILES = cfg.CORES, cfg.NPC, cfg.RPC, cfg.TILES
    import ml_dtypes
    iota_np = np.broadcast_to(np.arange(2 * P).astype(ml_dtypes.bfloat16),
                              (P, 2 * P)).copy()
    in_maps = []
    for c in range(CORES):
        sl = slice(c * NPC, (c + 1) * NPC)
        xT = np.zeros((D, RPC), np.float32)
        xT[:, :NPC] = x[sl].T
        d1c = np.zeros(RPC, np.float32)
        d1c[:NPC] = dinv[sl]
        d1 = d1c.reshape(TILES, P).T.copy()
        d2 = (d1 * d1).astype(np.float32)
        im = {
            "xT": xT.astype(ml_dtypes.bfloat16),
            "dinv1": d1,
            "dinv2": d2,
            "iota": iota_np,
            "idx16": per_core[c]["idx16"],
            "dstrel": per_core[c]["dstrel"],
        }
        for i in range(3):
            im[f"W{i + 1}"] = Ws[i]
            im[f"bias{i + 1}"] = np.broadcast_to(
                bss[i], (P, D)).astype(np.float32).copy()
        in_maps.append(im)
    return in_maps


_CACHE = {}


def kernel(x, edge_index, W1, b1, W2, b2, W3, b3):
    cfg = DEFAULT_CFG
    x = np.asarray(x, dtype=np.float32)
    Ws = [np.asarray(w, dtype=np.float32) for w in (W1, W2, W3)]
    bss = [np.asarray(b, dtype=np.float32) for b in (b1, b2, b3)]

    ei = np.asarray(edge_index)
    key = hash(ei[:, ::997].tobytes()) ^ hash(ei.shape)
    if key not in _CACHE:
        meta, per_core = _preprocess(ei, cfg)
        nc = _build_program(meta, cfg)
        _CACHE[key] = (meta, per_core, nc)
    meta, per_core, nc = _CACHE[key]

    in_maps = make_in_maps(x, Ws, bss, meta, per_core, cfg)
    res = run_bass_kernel_spmd(nc, in_maps, core_ids=list(range(cfg.CORES)))
    out = np.empty((cfg.N, D), np.float32)
    for c in range(cfg.CORES):
        out[c * cfg.NPC:(c + 1) * cfg.NPC] = res.results[c]["out"][:cfg.NPC]
    return out


# revision 27
# speedup vs baseline: 1.0339x; 1.0272x over previous
"""3-layer GCN (GCNConv x3 + LeakyReLU, PyG semantics) on 8 Trainium2 cores.

Strategy (source-partitioned, ReduceScatter):
  - Core c owns nodes [c*NPC, (c+1)*NPC) and processes the edges whose SOURCE
    it owns.  Self loops are added as explicit edges, so a whole layer is
        OUT[d] = lrelu(dinv[d] * sum_{e: dst=d} G[src_e] + bias),
    with G = dinv * (H @ W) computed locally per core (no feature exchange).
  - Per layer: phase A computes G for own nodes into two local DRAM gather
    tables (low/high half of own rows, int16-indexable, 256B rows), then
    dma_gather over globally dst-sorted edges + one-hot matmul segment-sum
    into PSUM per global dst tile (784 tiles across all cores), converted to
    bf16 and written into a row-major [100352, 64] partial-sum table.
  - One ReduceScatter(add) delivers each core the full sum for its own
    12544 rows -- an output-sized collective (~57us) instead of AllGathering
    the whole feature table (~284us).
  - Epilogue per own tile: scale by dinv (ACT), add bias (DVE), LeakyReLU
    (ACT), PE transpose, and the next layer's phase-A matmul (bf16).
  - One-hot q matrices (is_equal(iota, drel)) are split across DVE and Pool;
    PSUM->bf16 converts across ACT/DVE, keeping every engine below the DMA
    roofline.  Gathers use one large call per (gather-group, table-half)
    span to amortize the ~1us SWDGE fixed cost per call.

The Bass program is SPMD: one program, per-core data.  Section lengths are
shared across cores (max over cores, padded with dummy edges whose one-hot
column is zero: dstrel = -1).
"""
import sys

sys.path.insert(0, "/opt/trn_rl_repo")

import numpy as np

import concourse.bacc as bacc
import concourse.mybir as mybir
import concourse.tile as tile
from concourse import library_config
from concourse.dve_ops import AFFINE_THEN_ADD
from concourse.bass_utils import run_bass_kernel_spmd
from concourse.masks import make_identity

_F32 = mybir.dt.float32
_BF16 = mybir.dt.bfloat16
_I16 = mybir.dt.int16
P = 128
D = 64
NEG_SLOPE = 0.01


class Cfg:
    def __init__(self, n_nodes=100000, cores=8):
        self.N = n_nodes
        self.CORES = cores
        self.NPC = self.N // cores            # nodes owned per core
        self.TILES = (self.NPC + P - 1) // P  # own-node tiles per core (98)
        self.RPC = self.TILES * P             # padded rows per core (12544)
        self.GR = cores * self.RPC            # global padded rows (100352)
        self.GTILES = cores * self.TILES      # global dst tiles (784)
        self.KB = 1                           # source buckets (gather tables)
        assert self.RPC % self.KB == 0
        self.HALF = self.RPC // self.KB       # rows per gather table (12544)
        assert self.HALF <= 32767
        self.GGT = 28                         # dst tiles per gather group
        assert self.GTILES % self.GGT == 0
        self.NGG = self.GTILES // self.GGT    # gather groups (28)
        self.GT = 7                           # dst tiles per write group
        assert self.GGT % self.GT == 0
        self.WPG = self.GGT // self.GT        # write groups per gather group
        self.HTILES = self.TILES // self.KB   # own tiles per table (98)
        assert self.TILES % self.KB == 0
        self.LAYERS = 3
        self.MSGBUFS = 3
        self.QBUFS = 12
        self.PSABUFS = 3
        # engine split knobs
        self.Q_SPLIT = 1000000      # every Q_SPLIT-th q op goes to Pool
        self.CONV_MOD = 1000000     # converts: i%MOD==0 -> DVE, else ACT


DEFAULT_CFG = Cfg()


def _preprocess(edge_index, cfg):
    """Sort/pack edges; build per-core device arrays and shared metadata."""
    src0 = np.asarray(edge_index[0], dtype=np.int64)
    dst0 = np.asarray(edge_index[1], dtype=np.int64)
    N, CORES, NPC, TILES = cfg.N, cfg.CORES, cfg.NPC, cfg.TILES
    NGG, KB, GGT, HALF = cfg.NGG, cfg.KB, cfg.GGT, cfg.HALF

    src = src0
    dst = dst0

    deg = np.bincount(dst, minlength=N).astype(np.float32) + 1.0
    dinv = (1.0 / np.sqrt(deg)).astype(np.float32)

    owner = src // NPC                      # processing core (source owner)
    slocal = src % NPC
    bucket = slocal // HALF                 # gather-table half
    lrow = (slocal % HALF).astype(np.int16)

    downer = dst // NPC
    dloc = dst % NPC
    otile = dloc // P                       # owner-local tile 0..97
    HT2 = TILES // 2
    # (half, owner, tile-in-half): ReduceScatter halves then split per owner
    gtile = (otile // HT2) * (CORES * HT2) + downer * HT2 + otile % HT2
    drel_v = (dloc % P).astype(np.float32)
    grp = gtile // GGT
    tl = gtile % GGT

    counts = np.zeros((CORES, NGG, KB, GGT), dtype=np.int64)
    np.add.at(counts, (owner, grp, bucket, tl), 1)
    order = np.lexsort((tl, bucket, grp, owner))
    s_lrow = lrow[order]
    s_drel = drel_v[order]

    sec_len = counts.max(axis=0)            # [NGG, KB, GGT] shared sections

    # layout: gather group -> span (padded to x128) -> tile sections.
    # Each 128-edge block gets one drel column per covered tile PAIR: edges
    # of the pair's second tile store dstrel+128 and the one-hot is built
    # with a single 256-wide is_equal against iota256.
    ginfos = []
    tot_blocks = 0
    tot_cols = 0
    for g in range(NGG):
        gi = {"blk0": tot_blocks, "col0": tot_cols, "spans": {}, "tb": {},
              "bcols": [], "tmm": {}}
        assert KB == 1
        b = 0
        span_len = int(sec_len[g, b].sum())
        kgb = -(-span_len // P)
        gi["spans"][b] = (0, kgb)
        so = 0
        sec_rng = []                    # (t, start, end) in span coords
        for t in range(GGT):
            stb = int(sec_len[g, b, t])
            if stb == 0:
                continue
            gi["tb"][(b, t)] = (0, 0, 0, so)
            sec_rng.append((t, so, so + stb))
            so += stb
        # per block: covered tiles -> pair columns
        gcol = 0
        bcols = []                      # per block: (colbase, t0, ntiles)
        for j in range(kgb):
            lo, hi = j * P, (j + 1) * P
            tl_cov = [t for (t, a, e) in sec_rng if a < hi and e > lo]
            if not tl_cov:
                bcols.append((gcol, -1, 0))
                continue
            t0 = tl_cov[0]
            nt = len(tl_cov)
            assert tl_cov == list(range(t0, t0 + nt))
            bcols.append((gcol, t0, nt))
            gcol += -(-nt // 2)
        gi["bcols"] = bcols
        # per tile: ordered (block, pair m, side) matmul list
        for (t, a, e) in sec_rng:
            lst = []
            for j in range(a // P, (e - 1) // P + 1):
                colbase, t0, nt = bcols[j]
                if t0 < 0:
                    continue
                m = (t - t0) // 2
                side = (t - t0) % 2
                lst.append((j, m, side))
            gi["tmm"][t] = lst
        gi["blocks"] = kgb
        gi["ncols"] = gcol
        tot_blocks += kgb
        tot_cols += gcol
        ginfos.append(gi)
    tot_idx = tot_blocks * P

    # per-core run starts in the sorted edge list ((c, g, b, t)-major order)
    flat = counts.reshape(-1)
    starts = np.zeros(flat.size, dtype=np.int64)
    starts[1:] = np.cumsum(flat)[:-1]
    starts = starts.reshape(CORES, NGG, KB, GGT)

    per_core = []
    for c in range(CORES):
        lidx_flat = np.zeros(tot_idx, dtype=np.int16)
        drel = np.full((P, tot_cols), -1.0, dtype=np.float32)
        for g in range(NGG):
            gi = ginfos[g]
            bcol_base = np.array([bc[0] for bc in gi["bcols"]], dtype=np.int64)
            bcol_t0 = np.array([bc[1] for bc in gi["bcols"]], dtype=np.int64)
            for b, (bo, kgb) in gi["spans"].items():
                span_i0 = (gi["blk0"] + bo) * P
                for t in range(GGT):
                    if (b, t) not in gi["tb"]:
                        continue
                    so = gi["tb"][(b, t)][3]
                    n = int(counts[c, g, b, t])
                    if n == 0:
                        continue
                    s0 = int(starts[c, g, b, t])
                    pos0 = span_i0 + so
                    lidx_flat[pos0:pos0 + n] = s_lrow[s0:s0 + n]
                    q = so + np.arange(n)
                    j = q // P
                    pp = (pos0 + np.arange(n)) % P
                    toff = t - bcol_t0[j]
                    cols = gi["col0"] + bcol_base[j] + toff // 2
                    drel[pp, cols] = s_drel[s0:s0 + n] + 128.0 * (toff % 2)
        idx16 = np.tile(lidx_flat.reshape(tot_idx // 16, 16).T, (8, 1)).copy()
        per_core.append({"idx16": idx16, "dstrel": drel})

    meta = {
        "ginfos": ginfos,
        "tot_idx": tot_idx,
        "tot_cols": tot_cols,
        "dinv": dinv,
    }
    return meta, per_core


def _build_program(meta, cfg):
    ginfos = meta["ginfos"]
    tot_idx = meta["tot_idx"]
    tot_cols = meta["tot_cols"]
    CORES, TILES, RPC = cfg.CORES, cfg.TILES, cfg.RPC
    NGG, KB, GGT, GT = cfg.NGG, cfg.KB, cfg.GGT, cfg.GT
    HALF, HTILES, WPG, GR = cfg.HALF, cfg.HTILES, cfg.WPG, cfg.GR
    HT2 = TILES // 2

    kmax_g = max(gi["blocks"] for gi in ginfos)

    nc = bacc.Bacc("TRN2", debug=False)
    nc.num_devices = CORES

    xT_in = nc.dram_tensor("xT", [D, RPC], _BF16, kind="ExternalInput")
    dinv1_in = nc.dram_tensor("dinv1", [P, TILES], _F32, kind="ExternalInput")
    dinv2_in = nc.dram_tensor("dinv2", [P, TILES], _F32, kind="ExternalInput")
    w_in = [nc.dram_tensor(f"W{i + 1}", [D, D], _F32, kind="ExternalInput")
            for i in range(3)]
    bias_in = [nc.dram_tensor(f"bias{i + 1}", [P, D], _F32,
                              kind="ExternalInput") for i in range(3)]
    iota_in = nc.dram_tensor("iota", [P, 2 * P], _BF16,
                            kind="ExternalInput")
    idx_in = nc.dram_tensor("idx16", [P, tot_idx // 16], _I16,
                            kind="ExternalInput")
    drel_in = nc.dram_tensor("dstrel", [P, tot_cols], _F32,
                             kind="ExternalInput")
    out_t = nc.dram_tensor("out", [RPC, D], _F32, kind="ExternalOutput")

    with tile.TileContext(nc) as tc:
        with tc.tile_pool(name="dram", bufs=1, space="DRAM") as dram, \
             tc.tile_pool(name="const", bufs=1) as cst, \
             tc.tile_pool(name="msgp", bufs=cfg.MSGBUFS) as msgp, \
             tc.tile_pool(name="qp", bufs=cfg.QBUFS) as qp, \
             tc.tile_pool(name="qpl", bufs=24) as qpl, \
             tc.tile_pool(name="gsp", bufs=2) as gsp, \
             tc.tile_pool(name="psp", bufs=3) as psp, \
             tc.tile_pool(name="rsp", bufs=2) as rsp, \
             tc.tile_pool(name="hsp", bufs=2) as hsp, \
             tc.tile_pool(name="wk", bufs=4) as wk, \
             tc.tile_pool(name="htp", bufs=4) as htpp, \
             tc.tile_pool(name="psa", bufs=cfg.PSABUFS, space="PSUM") as psa, \
             tc.tile_pool(name="psg", bufs=2, space="PSUM") as psg, \
             tc.tile_pool(name="pst", bufs=2, space="PSUM") as pst:

            nc.gpsimd.load_library(library_config.mlp)

            gtabs = [[dram.tile([HALF, 2 * D], _BF16, name=f"gt{L}_{b}")
                      for b in range(KB)] for L in range(cfg.LAYERS)]
            partials = [[dram.tile([GR // 2, D], _BF16, name=f"part{L}_{h}")
                         for h in range(2)] for L in range(cfg.LAYERS)]
            rsouts = [[dram.tile([RPC // 2, D], _BF16,
                                 name=f"rsout{L}_{h}") for h in range(2)]
                      for L in range(cfg.LAYERS)]

            iota = cst.tile([P, 2 * P], _BF16)
            nc.sync.dma_start(iota[:], iota_in[:])
            ident = cst.tile([P, P], _F32)
            make_identity(nc, ident[:])
            dinv1 = cst.tile([P, TILES], _F32)
            nc.sync.dma_start(dinv1[:], dinv1_in[:])
            dinv2 = cst.tile([P, TILES], _F32)
            nc.sync.dma_start(dinv2[:], dinv2_in[:])
            g2b = cst.tile([P, TILES * D], _BF16)
            ws, bs = [], []
            for i in range(3):
                w = cst.tile([D, D], _F32, name=f"w{i}")
                nc.sync.dma_start(w[:], w_in[i][:])
                wb = cst.tile([D, D], _BF16, name=f"wb{i}")
                nc.vector.tensor_copy(wb[:], w[:])
                ws.append(wb)
                bt = cst.tile([P, D], _F32, name=f"b{i}")
                nc.sync.dma_start(bt[:], bias_in[i][:])
                bs.append(bt)
            ht0 = cst.tile([D, RPC], _BF16)        # layer-0 input (x.T)
            nc.sync.dma_start(ht0[:, :RPC // 2], xT_in[:, :RPC // 2])
            nc.sync.dma_start(ht0[:, RPC // 2:], xT_in[:, RPC // 2:])



            idx_sb = cst.tile([P, tot_idx // 16], _I16)
            idx_cut = (ginfos[4]["blk0"] * P) // 16 if NGG > 4 else 0
            if idx_cut:
                nc.sync.dma_start(idx_sb[:, :idx_cut], idx_in[:, :idx_cut])
            drel_sb = cst.tile([P, tot_cols], _F32)
            nc.sync.dma_start(drel_sb[:], drel_in[:])
            if idx_cut:
                nc.sync.dma_start(idx_sb[:, idx_cut:], idx_in[:, idx_cut:])
            else:
                nc.sync.dma_start(idx_sb[:], idx_in[:])

            qctr = [0]
            cctr = [0]

            def build_q(qt, col, width):
                eng = nc.vector
                if width <= P:
                    if qctr[0] % cfg.Q_SPLIT == cfg.Q_SPLIT - 1:
                        eng = nc.gpsimd
                    qctr[0] += 1
                eng.tensor_scalar(
                    out=qt[:, :width], in0=iota[:, :width],
                    scalar1=drel_sb[:, col:col + 1], scalar2=None,
                    op0=mybir.AluOpType.is_equal)
                return eng

            def convert(dst_ap, src_ap):
                m = cctr[0] % cfg.CONV_MOD
                cctr[0] += 1
                if m == 0:
                    nc.vector.tensor_copy(dst_ap, src_ap)
                else:
                    nc.scalar.copy(dst_ap, src_ap)

            def emit_gathers(L, g):
                gi = ginfos[g]
                msg = msgp.tile([P, gi["blocks"], 2 * D], _BF16,
                                tag="msg", name=f"msg{L}_{g}",
                                padded_shape=[P, kmax_g, 2 * D])
                for b, (bo, kgb) in gi["spans"].items():
                    i0 = (gi["blk0"] + bo) * P
                    nidx = kgb * P
                    nc.gpsimd.dma_gather(
                        msg[:, bo:bo + kgb, :],
                        gtabs[L][b][:],
                        idx_sb[:, i0 // 16:(i0 + nidx) // 16],
                        nidx, nidx, 2 * D,
                        single_packet=False)
                return msg

            def emit_process(L, g, msg):
                gi = ginfos[g]
                bcols = gi["bcols"]
                qtiles = {}                      # (j, m) -> q tile

                def get_q(j, m):
                    key = (j, m)
                    if key not in qtiles:
                        colbase, t0, nt = bcols[j]
                        width = 2 * P if (nt - 2 * m) >= 2 else P
                        if (width <= P
                                and qctr[0] % cfg.Q_SPLIT == cfg.Q_SPLIT - 1):
                            qt = qpl.tile([P, P], _BF16, tag="qpl",
                                          name=f"q{L}_{g}_{j}_{m}")
                        else:
                            qt = qp.tile([P, 2 * P], _BF16, tag="q",
                                         name=f"q{L}_{g}_{j}_{m}")
                        build_q(qt, gi["col0"] + colbase + m, width)
                        qtiles[key] = qt
                    return qtiles[key]

                for wg in range(WPG):
                    pstage = psp.tile([P, GT * D], _BF16, tag="ps",
                                      name=f"ps{L}_{g}_{wg}")
                    pa = psa.tile([P, GT * D], _F32, tag="pa",
                                  name=f"pa{L}_{g}_{wg}")
                    for tw in range(GT):
                        t = wg * GT + tw
                        tmm = gi["tmm"].get(t, [])
                        nmm = len(tmm)
                        assert nmm > 0
                        for done, (j, m, side) in enumerate(tmm):
                            qt = get_q(j, m)
                            nc.tensor.matmul(
                                pa[:, tw * D:(tw + 1) * D],
                                lhsT=qt[:, side * P:(side + 1) * P],
                                rhs=msg[:, j, :D],
                                start=(done == 0),
                                stop=(done == nmm - 1))
                    convert(pstage[:], pa[:])
                    gr = g * GGT + wg * GT          # global tile index
                    half = gr // (CORES * HT2)
                    r0 = (gr - half * CORES * HT2) * P
                    dst = partials[L][half][r0:r0 + GT * P, :].rearrange(
                        "(j p) c -> p j c", j=GT, p=P)
                    nc.sync.dma_start(dst, pstage[:].rearrange(
                        "p (j c) -> p j c", j=GT, c=D))

            def emit_phase_a0():
                """Layer-0 phase A from ht0 (no epilogue)."""
                for half in range(2):
                    gstage = gsp.tile([P, HT2 * 2 * D], _BF16, tag="gs",
                                      name=f"gs0_{half}")
                    nc.vector.memset(gstage[:], 0.0)
                    for th in range(HT2):
                        t = half * HT2 + th
                        pg = psg.tile([P, D], _F32, tag="pg",
                                      name=f"pg0_{t}")
                        nc.tensor.matmul(pg[:],
                                         lhsT=ht0[:, t * P:(t + 1) * P],
                                         rhs=ws[0][:], start=True, stop=True)
                        nc.vector.tensor_scalar_mul(
                            gstage[:, th * 2 * D:th * 2 * D + D],
                            pg[:], dinv1[:, t:t + 1])
                        g2 = wk.tile([P, D], _F32, tag="g2",
                                     name=f"g2_0_{t}")
                        nc.scalar.mul(g2[:], pg[:], dinv2[:, t:t + 1])
                        nc.vector.tensor_tensor(
                            out=g2b[:, t * D:(t + 1) * D], in0=g2[:],
                            in1=bs[0][:], op=mybir.AluOpType.add)
                    r0 = half * HT2 * P
                    dst = gtabs[0][0][r0:r0 + HT2 * P, :].rearrange(
                        "(j p) c -> p j c", j=HT2, p=P)
                    nc.sync.dma_start(dst, gstage[:].rearrange(
                        "p (j c) -> p j c", j=HT2, c=2 * D))

            def make_half_closures(L, half):
                """Closures: epilogue of layer L-1 results for own tiles of
                `half`, fused with phase A of layer L (or the final output
                when L == LAYERS).  Call in order; interleavable."""
                final = (L == cfg.LAYERS)
                state = {}

                def load_rs():
                    rsall = rsp.tile([P, HT2 * D], _BF16, tag="rs",
                                     name=f"rsall{L}_{half}")
                    src = rsouts[L - 1][half][:].rearrange(
                        "(j p) c -> p j c", j=HT2, p=P)
                    nc.scalar.dma_start(rsall[:].rearrange(
                        "p (j c) -> p j c", j=HT2, c=D), src)
                    state["rs"] = rsall
                    if final:
                        state["hs"] = hsp.tile([P, HT2 * D], _F32, tag="hs",
                                               name=f"hs_{half}")
                    else:
                        gst = gsp.tile([P, HT2 * 2 * D], _BF16, tag="gs",
                                       name=f"gs{L}_{half}")
                        state["gs"] = gst

                def tile_work(th):
                    t = half * HT2 + th
                    rsall = state["rs"]
                    v = wk.tile([P, D], _BF16, tag="v", name=f"v{L}_{t}")
                    nc.vector._custom_dve(
                        AFFINE_THEN_ADD, out=v[:],
                        in0=rsall[:, th * D:(th + 1) * D],
                        in1=g2b[:, t * D:(t + 1) * D],
                        s0=dinv1[:, t:t + 1], s1=0.0, imm2=0.0)
                    if final:
                        nc.scalar.activation(
                            state["hs"][:, th * D:(th + 1) * D], v[:],
                            mybir.ActivationFunctionType.Lrelu,
                            bias=0.0, scale=1.0, alpha=NEG_SLOPE)
                        return
                    h = wk.tile([P, D], _F32, tag="h", name=f"h{L}_{t}")
                    nc.scalar.activation(
                        h[:], v[:], mybir.ActivationFunctionType.Lrelu,
                        bias=0.0, scale=1.0, alpha=NEG_SLOPE)
                    pt = pst.tile([D, P], _F32, tag="pt", name=f"pt{L}_{t}")
                    nc.tensor.transpose(pt[:], h[:], ident[:])
                    htp = htpp.tile([D, P], _BF16, tag="ht",
                                    name=f"htp{L}_{t}")
                    if half == 0:
                        nc.scalar.copy(htp[:], pt[:])
                    else:
                        nc.vector.tensor_copy(htp[:], pt[:])
                    pg = psg.tile([P, D], _F32, tag="pg", name=f"pg{L}_{t}")
                    nc.tensor.matmul(pg[:], lhsT=htp[:], rhs=ws[L][:],
                                     start=True, stop=True)
                    gslice = state["gs"][:, th * 2 * D:th * 2 * D + D]
                    if half == 0:
                        nc.scalar.mul(gslice, pg[:], dinv1[:, t:t + 1])
                    else:
                        nc.vector.tensor_scalar_mul(gslice, pg[:],
                                                    dinv1[:, t:t + 1])
                    g2 = wk.tile([P, D], _F32, tag="g2", name=f"g2_{L}_{t}")
                    nc.scalar.mul(g2[:], pg[:], dinv2[:, t:t + 1])
                    nc.vector.tensor_tensor(
                        out=g2b[:, t * D:(t + 1) * D], in0=g2[:],
                        in1=bs[L][:], op=mybir.AluOpType.add)

                def flush():
                    r0 = half * HT2 * P
                    if final:
                        dst = out_t[r0:r0 + HT2 * P, :].rearrange(
                            "(j p) c -> p j c", j=HT2, p=P)
                        nc.sync.dma_start(dst, state["hs"][:].rearrange(
                            "p (j c) -> p j c", j=HT2, c=D))
                    else:
                        dst = gtabs[L][0][r0:r0 + HT2 * P, :].rearrange(
                            "(j p) c -> p j c", j=HT2, p=P)
                        nc.sync.dma_start(dst, state["gs"][:].rearrange(
                            "p (j c) -> p j c", j=HT2, c=2 * D))

                return ([load_rs] + [lambda th=th: tile_work(th)
                                     for th in range(HT2)] + [flush])

            def emit_rs(L, half):
                nc.gpsimd.collective_compute(
                    "ReduceScatter",
                    mybir.AluOpType.add,
                    replica_groups=[list(range(CORES))],
                    ins=[partials[L][half][:]],
                    outs=[rsouts[L][half][:]],
                )

            NHALF = NGG // 2
            ILV_START = NHALF + 5       # first group that interleaves epi

            for L in range(cfg.LAYERS):
                if L == 0:
                    emit_phase_a0()
                msgs = {}
                for gg in range(min(3, NGG)):
                    msgs[gg] = emit_gathers(L, gg)
                pend = []
                for g in range(NGG):
                    if g + 3 < NGG:
                        msgs[g + 3] = emit_gathers(L, g + 3)
                    emit_process(L, g, msgs.pop(g))
                    if g == NHALF + 2:
                        emit_rs(L, 0)
                        pend = make_half_closures(L + 1, 0)
                    if g >= ILV_START and pend:
                        take = -(-len(pend) // (NGG - g))
                        for fn in pend[:take]:
                            fn()
                        pend = pend[take:]
                for fn in pend:
                    fn()
                emit_rs(L, 1)
                for fn in make_half_closures(L + 1, 1):
                    fn()

    nc.compile()
    return nc


def make_in_maps(x, Ws, bss, meta, per_core, cfg):
    dinv = meta["dinv"]
    CORES, NPC, RPC, TILES = cfg.CORES, cfg.NPC, cfg.RPC, cfg.TILES
    import ml_dtypes
    iota_np = np.broadcast_to(np.arange(2 * P).astype(ml_dtypes.bfloat16),
                              (P, 2 * P)).copy()
    in_maps = []
    for c in range(CORES):
        sl = slice(c * NPC, (c + 1) * NPC)
        xT = np.zeros((D, RPC), np.float32)
        xT[:, :NPC] = x[sl].T
        d1c = np.zeros(RPC, np.float32)
        d1c[:NPC] = dinv[sl]
        d1 = d1c.reshape(TILES, P).T.copy()
        d2 = (d1 * d1).astype(np.float32)
        im = {
            "xT": xT.astype(ml_dtypes.bfloat16),
            "dinv1": d1,
            "dinv2": d2,
            "iota": iota_np,
            "idx16": per_core[c]["idx16"],
            "dstrel": per_core[c]["dstrel"],
        }
        for i in range(3):
            im[f"W{i + 1}"] = Ws[i]
            im[f"bias{i + 1}"] = np.broadcast_to(
                bss[i], (P, D)).astype(np.float32).copy()
        in_maps.append(im)
    return in_maps


_CACHE = {}


def kernel(x, edge_index, W1, b1, W2, b2, W3, b3):
    cfg = DEFAULT_CFG
    x = np.asarray(x, dtype=np.float32)
    Ws = [np.asarray(w, dtype=np.float32) for w in (W1, W2, W3)]
    bss = [np.asarray(b, dtype=np.float32) for b in (b1, b2, b3)]

    ei = np.asarray(edge_index)
    key = hash(ei[:, ::997].tobytes()) ^ hash(ei.shape)
    if key not in _CACHE:
        meta, per_core = _preprocess(ei, cfg)
        nc = _build_program(meta, cfg)
        _CACHE[key] = (meta, per_core, nc)
    meta, per_core, nc = _CACHE[key]

    in_maps = make_in_maps(x, Ws, bss, meta, per_core, cfg)
    res = run_bass_kernel_spmd(nc, in_maps, core_ids=list(range(cfg.CORES)))
    out = np.empty((cfg.N, D), np.float32)
    for c in range(cfg.CORES):
        out[c * cfg.NPC:(c + 1) * cfg.NPC] = res.results[c]["out"][:cfg.NPC]
    return out
